# Initial kernel scaffold
#
"""HNHN layer (hypergraph message passing) on 8 Trainium2 NeuronCores.

Math (reference, with B1 the binary node-edge incidence matrix):
    edge_card = colsum(B1)^-1.5          node_card = rowsum(B1)^-0.5
    d0_inv    = 1/(B1 @ edge_card)       d1_inv    = 1/(B1^T @ node_card)
    x1     = d1_inv * (B1^T @ (node_card * (x0 @ W0))) + b01
    x0_out = d0_inv * (B1  @ (edge_card * (x1 @ W1))) + b10
    returns (relu(x0_out), relu(x1))

Implementation: nodes are row-sharded 8 ways. Per core, three streaming
passes over the (bf16, exact for 0/1 values) incidence shard:
  P1: row sums (DVE reduce)  -> node_card
  P2: U = B1_shard^T @ [node_card*y | node_card | 1]   (PE, psum-accum)
      -> ReduceScatter(add) over cores gives each core its edge shard of
         [B1^T Yp | d1 | colsums]
  P3: V = B1_shard @ [edge_card*z | edge_card]          (PE, psum-accum)
      where z = x1_shard @ W1 is computed on the edge shard and
      AllGather'ed (bf16) to every core.
Stats ride along as extra matmul columns so no separate matvec passes are
needed. Only host-side prep: sharding, transposes, bf16 casts.
"""

import numpy as np
import ml_dtypes

import concourse.bass as bass
import concourse.mybir as mybir
import concourse.tile as tile
from concourse import bacc
from concourse.bass_utils import run_bass_kernel_spmd
from concourse.masks import make_identity

BF16 = ml_dtypes.bfloat16

N_NODES, N_EDGES = 16384, 8192
CH = 256
N_CORES = 8
NS = N_NODES // N_CORES      # 2048 nodes per core
ES = N_EDGES // N_CORES      # 1024 edges per core (edge shard)
P = 128
NB = NS // P                 # 16 node blocks / core
EBF = N_EDGES // P           # 64 edge blocks (full)
EBS = ES // P                # 8 edge blocks (shard)
F32 = mybir.dt.float32
BF = mybir.dt.bfloat16
AX = mybir.AxisListType
ALU = mybir.AluOpType
GROUPS = [list(range(N_CORES))]


def build_bass():
    nc = bacc.Bacc("TRN2", target_bir_lowering=False, debug=False,
                   num_devices=N_CORES)
    b1r = nc.dram_tensor("b1r", [NS, N_EDGES], BF, kind="ExternalInput")
    b1t = nc.dram_tensor("b1t", [N_EDGES, NS], BF, kind="ExternalInput")
    x0t = nc.dram_tensor("x0t", [CH, NS], F32, kind="ExternalInput")
    w0 = nc.dram_tensor("w0", [CH, CH], F32, kind="ExternalInput")
    w1 = nc.dram_tensor("w1", [CH, CH], F32, kind="ExternalInput")
    b01 = nc.dram_tensor("b01", [1, CH], F32, kind="ExternalInput")
    b10 = nc.dram_tensor("b10", [1, CH], F32, kind="ExternalInput")
    x0o = nc.dram_tensor("x0o", [NS, CH], F32, kind="ExternalOutput")
    x1o = nc.dram_tensor("x1o", [ES, CH], F32, kind="ExternalOutput")

    with tile.TileContext(nc) as tc:
        with (
            tc.tile_pool(name="const", bufs=1) as const,
            tc.tile_pool(name="psum", bufs=8, space="PSUM") as psum,
            tc.tile_pool(name="dram", bufs=1, space="DRAM") as dram,
            tc.tile_pool(name="small", bufs=4) as small,
            tc.tile_pool(name="evac", bufs=4) as evac,
        ):
            # ---- constants / persistent tensors ----
            x0t_sb = const.tile([P, 2, NS], F32)
            nc.sync.dma_start(x0t_sb[:], x0t.rearrange("(k p) n -> p k n", p=P))
            w0_sb = const.tile([P, 2, CH], F32)
            nc.sync.dma_start(w0_sb[:], w0.rearrange("(k p) c -> p k c", p=P))
            w1_sb = const.tile([P, 2, CH], F32)
            nc.sync.dma_start(w1_sb[:], w1.rearrange("(k p) c -> p k c", p=P))
            w1bf_sb = const.tile([P, 2, CH], BF)
            nc.vector.tensor_copy(w1bf_sb[:], w1_sb[:])
            b01_sb = const.tile([1, CH], F32)
            nc.sync.dma_start(b01_sb[:], b01[:])
            b10_sb = const.tile([1, CH], F32)
            nc.sync.dma_start(b10_sb[:], b10[:])
            ident = const.tile([P, P], F32)
            make_identity(nc, ident[:])

            y_sb = const.tile([P, NB, CH], F32)       # x0 @ W0 (node shard)
            yp_sb = const.tile([P, NB, CH + 2], BF)   # [nc*y | nc | 1]
            x1t_sb = const.tile([P, 2, ES], BF)       # x1 shard, transposed
            zaug_sb = const.tile([P, EBF, CH + 1], BF)  # [ec*z | ec], all edges
            ecs_sb = const.tile([P, EBS], F32)        # edge_card per shard blk

            # ---- P0: y = x0 @ W0 ----
            for n in range(NB):
                ps = psum.tile([P, CH], F32, tag="ps", name=f"ps_y{n}")
                for k in range(2):
                    nc.tensor.matmul(ps[:], x0t_sb[:, k, n * P:(n + 1) * P],
                                     w0_sb[:, k, :], start=(k == 0),
                                     stop=(k == 1))
                nc.scalar.copy(y_sb[:, n, :], ps[:])

            # ---- P1: row sums -> node_card -> Yp ----
            with tc.tile_pool(name="rows", bufs=3) as rows:
                for n in range(NB):
                    rt = rows.tile([P, N_EDGES], BF, tag="rt", name=f"rt{n}")
                    nc.sync.dma_start(rt[:], b1r[n * P:(n + 1) * P, :])
                    rs = small.tile([P, 1], F32, tag="rs", name=f"rs{n}")
                    nc.vector.reduce_sum(rs[:], rt[:], axis=AX.X)
                    ri = small.tile([P, 1], F32, tag="ri", name=f"ri{n}")
                    nc.vector.reciprocal(ri[:], rs[:])
                    ncard = small.tile([P, 1], F32, tag="ncard", name=f"nc{n}")
                    nc.scalar.sqrt(ncard[:], ri[:])
                    nc.vector.tensor_scalar_mul(
                        yp_sb[:, n, 0:CH], y_sb[:, n, :], ncard[:])
                    nc.vector.tensor_copy(yp_sb[:, n, CH:CH + 1], ncard[:])
                    nc.vector.memset(yp_sb[:, n, CH + 1:CH + 2], 1.0)

            # ---- P2: U = B1^T @ Yp, accumulated over node blocks ----
            u_dram = dram.tile([N_EDGES, CH + 2], F32)
            with tc.tile_pool(name="octp", bufs=2) as octp:
                for oct_ in range(8):
                    ot = octp.tile([P, NB, 1024], BF, tag="oct",
                                   name=f"oct{oct_}")
                    for n in range(NB):
                        nc.sync.dma_start(
                            ot[:, n, :],
                            b1r[n * P:(n + 1) * P,
                                oct_ * 1024:(oct_ + 1) * 1024])
                    for e8 in range(8):
                        e = oct_ * 8 + e8
                        ups = psum.tile([P, CH + 2], F32, tag="ps",
                                        name=f"ps_u{e}")
                        for n in range(NB):
                            nc.tensor.matmul(
                                ups[:], ot[:, n, e8 * P:(e8 + 1) * P],
                                yp_sb[:, n, :],
                                start=(n == 0), stop=(n == NB - 1))
                        ue = evac.tile([P, CH + 2], F32, tag="ue",
                                       name=f"ue{e}")
                        nc.vector.tensor_copy(ue[:], ups[:])
                        nc.sync.dma_start(u_dram[e * P:(e + 1) * P, :], ue[:])

            # ---- C1: ReduceScatter U; per-shard tail -> x1, z, Zaug ----
            urs = dram.tile([ES, CH + 2], F32)
            nc.gpsimd.collective_compute(
                "ReduceScatter", ALU.add, replica_groups=GROUPS,
                ins=[u_dram[:].opt()], outs=[urs[:].opt()])

            zloc = dram.tile([ES, CH + 1], BF)
            for eb in range(EBS):
                ut = evac.tile([P, CH + 2], F32, tag="ut", name=f"ut{eb}")
                nc.sync.dma_start(ut[:], urs[eb * P:(eb + 1) * P, :])
                d1i = small.tile([P, 1], F32, tag="d1i", name=f"d1i{eb}")
                nc.vector.reciprocal(d1i[:], ut[:, CH:CH + 1])
                ci = small.tile([P, 1], F32, tag="ci", name=f"ci{eb}")
                nc.vector.reciprocal(ci[:], ut[:, CH + 1:CH + 2])
                cs = small.tile([P, 1], F32, tag="cs", name=f"cs{eb}")
                nc.scalar.sqrt(cs[:], ci[:])
                nc.vector.tensor_tensor(ecs_sb[:, eb:eb + 1], ci[:], cs[:],
                                        ALU.mult)
                x1 = evac.tile([P, CH], F32, tag="x1", name=f"x1_{eb}")
                nc.vector.tensor_scalar_mul(x1[:], ut[:, 0:CH], d1i[:])
                nc.vector.tensor_add(x1[:], x1[:],
                                     b01_sb[:].to_broadcast((P, CH)))
                x1r = evac.tile([P, CH], F32, tag="x1r", name=f"x1r{eb}")
                nc.vector.tensor_scalar_max(x1r[:], x1[:], 0.0)
                nc.sync.dma_start(x1o[eb * P:(eb + 1) * P, :], x1r[:])
                for c2 in range(2):
                    tp = psum.tile([P, P], F32, tag="ps", name=f"tp{eb}_{c2}")
                    nc.tensor.transpose(tp[:], x1[:, c2 * P:(c2 + 1) * P],
                                        ident[:])
                    nc.vector.tensor_copy(
                        x1t_sb[:, c2, eb * P:(eb + 1) * P], tp[:])
            for eb in range(EBS):
                zps = psum.tile([P, CH], F32, tag="ps", name=f"ps_z{eb}")
                for c2 in range(2):
                    nc.tensor.matmul(zps[:], x1t_sb[:, c2, eb * P:(eb + 1) * P],
                                     w1bf_sb[:, c2, :], start=(c2 == 0),
                                     stop=(c2 == 1))
                zt = evac.tile([P, CH + 1], BF, tag="zt", name=f"zt{eb}")
                nc.vector.tensor_scalar_mul(zt[:, 0:CH], zps[:],
                                            ecs_sb[:, eb:eb + 1])
                nc.vector.tensor_copy(zt[:, CH:CH + 1], ecs_sb[:, eb:eb + 1])
                nc.sync.dma_start(zloc[eb * P:(eb + 1) * P, :], zt[:])

            # ---- C2: AllGather Zaug (bf16) ----
            zaug_dram = dram.tile([N_EDGES, CH + 1], BF, addr_space="Shared")
            nc.gpsimd.collective_compute(
                "AllGather", ALU.bypass, replica_groups=GROUPS,
                ins=[zloc[:].opt()], outs=[zaug_dram[:].opt()])
            nc.sync.dma_start(
                zaug_sb[:], zaug_dram.rearrange("(e p) c -> p e c", p=P))

            # ---- P3: V = B1 @ Zaug, accumulated over edge blocks ----
            with tc.tile_pool(name="btp", bufs=8) as btp:
                for nh in range(2):
                    vps = []
                    for n8 in range(8):
                        vps.append(psum.tile([P, CH + 1], F32, tag="ps",
                                             name=f"ps_v{nh}_{n8}"))
                    for ebi in range(EBF):
                        bt = btp.tile([P, 1024], BF, tag="bt",
                                      name=f"bt{nh}_{ebi}")
                        nc.sync.dma_start(
                            bt[:], b1t[ebi * P:(ebi + 1) * P,
                                       nh * 1024:(nh + 1) * 1024])
                        for n8 in range(8):
                            nc.tensor.matmul(
                                vps[n8][:], bt[:, n8 * P:(n8 + 1) * P],
                                zaug_sb[:, ebi, :],
                                start=(ebi == 0), stop=(ebi == EBF - 1))
                    for n8 in range(8):
                        n = nh * 8 + n8
                        d0i = small.tile([P, 1], F32, tag="d0i",
                                         name=f"d0i{n}")
                        nc.vector.reciprocal(d0i[:], vps[n8][:, CH:CH + 1])
                        xo = evac.tile([P, CH], F32, tag="xo", name=f"xo{n}")
                        nc.vector.tensor_scalar_mul(xo[:], vps[n8][:, 0:CH],
                                                    d0i[:])
                        nc.vector.tensor_add(xo[:], xo[:],
                                             b10_sb[:].to_broadcast((P, CH)))
                        nc.vector.tensor_scalar_max(xo[:], xo[:], 0.0)
                        nc.sync.dma_start(x0o[n * P:(n + 1) * P, :], xo[:])

    nc.compile()
    return nc


_NC_CACHE = None


def _get_nc():
    global _NC_CACHE
    if _NC_CACHE is None:
        _NC_CACHE = build_bass()
    return _NC_CACHE


def kernel(x_0, incidence_1, W0, W1, bias_0_to_1, bias_1_to_0):
    x_0 = np.asarray(x_0, dtype=np.float32)
    b1 = np.asarray(incidence_1, dtype=np.float32)
    W0 = np.asarray(W0, dtype=np.float32)
    W1 = np.asarray(W1, dtype=np.float32)
    b01 = np.asarray(bias_0_to_1, dtype=np.float32).reshape(1, CH)
    b10 = np.asarray(bias_1_to_0, dtype=np.float32).reshape(1, CH)

    b1_bf = b1.astype(BF16)
    in_maps = []
    for i in range(N_CORES):
        rows = slice(i * NS, (i + 1) * NS)
        shard = b1_bf[rows]
        in_maps.append({
            "b1r": np.ascontiguousarray(shard),
            "b1t": np.ascontiguousarray(shard.T),
            "x0t": np.ascontiguousarray(x_0[rows].T),
            "w0": W0, "w1": W1, "b01": b01, "b10": b10,
        })

    nc = _get_nc()
    res = run_bass_kernel_spmd(nc, in_maps, core_ids=list(range(N_CORES)))
    x0_out = np.concatenate([r["x0o"] for r in res.results], axis=0)
    x1_out = np.concatenate([r["x1o"] for r in res.results], axis=0)
    return x0_out, x1_out


# revision 10
# speedup vs baseline: 1.1145x; 1.1145x over previous
"""HNHN layer (hypergraph message passing) on 8 Trainium2 NeuronCores.

Math (reference, with B1 the binary node-edge incidence matrix):
    edge_card = colsum(B1)^-1.5          node_card = rowsum(B1)^-0.5
    d0_inv    = 1/(B1 @ edge_card)       d1_inv    = 1/(B1^T @ node_card)
    x1     = d1_inv * (B1^T @ (node_card * (x0 @ W0))) + b01
    x0_out = d0_inv * (B1  @ (edge_card * (x1 @ W1))) + b10
    returns (relu(x0_out), relu(x1))

Implementation: nodes are row-sharded 8 ways. Per core, three streaming
passes over the (bf16, exact for 0/1 values) incidence shard:
  P1: row sums (DVE reduce)  -> node_card
  P2: U = B1_shard^T @ [node_card*y | node_card | 1]   (PE, psum-accum)
      -> ReduceScatter(add) over cores gives each core its edge shard of
         [B1^T Yp | d1 | colsums]
  P3: V = B1_shard @ [edge_card*z | edge_card]          (PE, psum-accum)
      where z = x1_shard @ W1 is computed on the edge shard and
      AllGather'ed (bf16) to every core.
Stats ride along as extra matmul columns so no separate matvec passes are
needed. Only host-side prep: sharding, transposes, bf16 casts.
"""

import numpy as np
import ml_dtypes

import concourse.bass as bass
import concourse.mybir as mybir
import concourse.tile as tile
from concourse import bacc
from concourse.bass_utils import run_bass_kernel_spmd
from concourse.masks import make_identity

BF16 = ml_dtypes.bfloat16

N_NODES, N_EDGES = 16384, 8192
CH = 256
N_CORES = 8
NS = N_NODES // N_CORES      # 2048 nodes per core
ES = N_EDGES // N_CORES      # 1024 edges per core (edge shard)
P = 128
NB = NS // P                 # 16 node blocks / core
EBF = N_EDGES // P           # 64 edge blocks (full)
EBS = ES // P                # 8 edge blocks (shard)
F32 = mybir.dt.float32
BF = mybir.dt.bfloat16
AX = mybir.AxisListType
ALU = mybir.AluOpType
GROUPS = [list(range(N_CORES))]


def build_bass(reps=1):
    nc = bacc.Bacc("TRN2", target_bir_lowering=False, debug=False,
                   num_devices=N_CORES)
    b1r = nc.dram_tensor("b1r", [NS, N_EDGES], BF, kind="ExternalInput")
    b1t = nc.dram_tensor("b1t", [N_EDGES, NS], BF, kind="ExternalInput")
    x0t = nc.dram_tensor("x0t", [CH, NS], F32, kind="ExternalInput")
    w0 = nc.dram_tensor("w0", [CH, CH], F32, kind="ExternalInput")
    w1 = nc.dram_tensor("w1", [CH, CH], F32, kind="ExternalInput")
    b01 = nc.dram_tensor("b01", [P, CH], F32, kind="ExternalInput")
    b10 = nc.dram_tensor("b10", [P, CH], F32, kind="ExternalInput")
    x0o = nc.dram_tensor("x0o", [NS, CH], F32, kind="ExternalOutput")
    x1o = nc.dram_tensor("x1o", [ES, CH], F32, kind="ExternalOutput")

    with tile.TileContext(nc) as tc:
        with (
            tc.tile_pool(name="const", bufs=1) as const,
            tc.tile_pool(name="psum", bufs=8, space="PSUM") as psum,
            tc.tile_pool(name="dram", bufs=1, space="DRAM") as dram,
            tc.tile_pool(name="small", bufs=4) as small,
            tc.tile_pool(name="evac", bufs=4) as evac,
        ):
            # ---- constants / persistent tensors ----
            x0t_sb = const.tile([P, 2, NS], F32)
            nc.sync.dma_start(x0t_sb[:], x0t.rearrange("(k p) n -> p k n", p=P))
            w0_sb = const.tile([P, 2, CH], F32)
            nc.sync.dma_start(w0_sb[:], w0.rearrange("(k p) c -> p k c", p=P))
            w1_sb = const.tile([P, 2, CH], F32)
            nc.sync.dma_start(w1_sb[:], w1.rearrange("(k p) c -> p k c", p=P))
            w1bf_sb = const.tile([P, 2, CH], BF)
            nc.vector.tensor_copy(w1bf_sb[:], w1_sb[:])
            b01_sb = const.tile([P, CH], F32)
            nc.sync.dma_start(b01_sb[:], b01[:])
            b10_sb = const.tile([P, CH], F32)
            nc.sync.dma_start(b10_sb[:], b10[:])
            ident = const.tile([P, P], F32)
            make_identity(nc, ident[:])

            y_sb = const.tile([P, NB, CH], F32)       # x0 @ W0 (node shard)
            yp_sb = const.tile([P, NB, CH + 2], BF)   # [nc*y | nc | 1]
            x1t_sb = const.tile([P, 2, ES], BF)       # x1 shard, transposed
            zaug_sb = const.tile([P, EBF, CH + 1], BF)  # [ec*z | ec], all edges
            ecs_sb = const.tile([P, EBS], F32)        # edge_card per shard blk

            for _rep in range(reps):
                _emit_body(nc, tc, psum, dram, small, evac, locals())

    nc.compile()
    return nc


def _emit_body(nc, tc, psum, dram, small, evac, env):
    b1r, b1t, x0o, x1o = env["b1r"], env["b1t"], env["x0o"], env["x1o"]
    x0t_sb, w0_sb, w1bf_sb = env["x0t_sb"], env["w0_sb"], env["w1bf_sb"]
    b01_sb, b10_sb, ident = env["b01_sb"], env["b10_sb"], env["ident"]
    y_sb, yp_sb, x1t_sb = env["y_sb"], env["yp_sb"], env["x1t_sb"]
    zaug_sb, ecs_sb = env["zaug_sb"], env["ecs_sb"]
    if True:
        if True:
            # ---- P0: y = x0 @ W0 ----
            for n in range(NB):
                ps = psum.tile([P, CH], F32, tag="ps", name=f"ps_y{n}")
                for k in range(2):
                    nc.tensor.matmul(ps[:], x0t_sb[:, k, n * P:(n + 1) * P],
                                     w0_sb[:, k, :], start=(k == 0),
                                     stop=(k == 1))
                nc.scalar.copy(y_sb[:, n, :], ps[:])

            # ---- P1: row sums -> node_card -> Yp ----
            with tc.tile_pool(name="rows", bufs=3) as rows:
                for n in range(NB):
                    rt = rows.tile([P, N_EDGES], BF, tag="rt", name=f"rt{n}")
                    nc.sync.dma_start(rt[:], b1r[n * P:(n + 1) * P, :])
                    rs = small.tile([P, 1], F32, tag="rs", name=f"rs{n}")
                    nc.vector.reduce_sum(rs[:], rt[:], axis=AX.X)
                    ri = small.tile([P, 1], F32, tag="ri", name=f"ri{n}")
                    nc.vector.reciprocal(ri[:], rs[:])
                    ncard = small.tile([P, 1], F32, tag="ncard", name=f"nc{n}")
                    nc.scalar.sqrt(ncard[:], ri[:])
                    nc.vector.tensor_scalar_mul(
                        yp_sb[:, n, 0:CH], y_sb[:, n, :], ncard[:])
                    nc.vector.tensor_copy(yp_sb[:, n, CH:CH + 1], ncard[:])
                    nc.vector.memset(yp_sb[:, n, CH + 1:CH + 2], 1.0)

            # ---- P2: U = B1^T @ Yp, accumulated over node blocks ----
            u_dram = dram.tile([N_EDGES, CH + 2], F32)
            with tc.tile_pool(name="octp", bufs=2) as octp:
                for oct_ in range(8):
                    ot = octp.tile([P, NB, 1024], BF, tag="oct",
                                   name=f"oct{oct_}")
                    for n in range(NB):
                        nc.sync.dma_start(
                            ot[:, n, :],
                            b1r[n * P:(n + 1) * P,
                                oct_ * 1024:(oct_ + 1) * 1024])
                    for e8 in range(8):
                        e = oct_ * 8 + e8
                        ups = psum.tile([P, CH + 2], F32, tag="ps",
                                        name=f"ps_u{e}")
                        for n in range(NB):
                            nc.tensor.matmul(
                                ups[:], ot[:, n, e8 * P:(e8 + 1) * P],
                                yp_sb[:, n, :],
                                start=(n == 0), stop=(n == NB - 1))
                        ue = evac.tile([P, CH + 2], F32, tag="ue",
                                       name=f"ue{e}")
                        nc.vector.tensor_copy(ue[:], ups[:])
                        nc.sync.dma_start(u_dram[e * P:(e + 1) * P, :], ue[:])

            # ---- C1: ReduceScatter U; per-shard tail -> x1, z, Zaug ----
            urs = dram.tile([ES, CH + 2], F32)
            nc.gpsimd.collective_compute(
                "ReduceScatter", ALU.add, replica_groups=GROUPS,
                ins=[u_dram[:].opt()], outs=[urs[:].opt()])

            zloc = dram.tile([ES, CH + 1], BF)
            for eb in range(EBS):
                ut = evac.tile([P, CH + 2], F32, tag="ut", name=f"ut{eb}")
                nc.sync.dma_start(ut[:], urs[eb * P:(eb + 1) * P, :])
                d1i = small.tile([P, 1], F32, tag="d1i", name=f"d1i{eb}")
                nc.vector.reciprocal(d1i[:], ut[:, CH:CH + 1])
                ci = small.tile([P, 1], F32, tag="ci", name=f"ci{eb}")
                nc.vector.reciprocal(ci[:], ut[:, CH + 1:CH + 2])
                cs = small.tile([P, 1], F32, tag="cs", name=f"cs{eb}")
                nc.scalar.sqrt(cs[:], ci[:])
                nc.vector.tensor_tensor(ecs_sb[:, eb:eb + 1], ci[:], cs[:],
                                        ALU.mult)
                x1 = evac.tile([P, CH], F32, tag="x1", name=f"x1_{eb}")
                nc.vector.tensor_scalar_mul(x1[:], ut[:, 0:CH], d1i[:])
                nc.vector.tensor_add(x1[:], x1[:], b01_sb[:])
                x1r = evac.tile([P, CH], F32, tag="x1r", name=f"x1r{eb}")
                nc.vector.tensor_scalar_max(x1r[:], x1[:], 0.0)
                nc.sync.dma_start(x1o[eb * P:(eb + 1) * P, :], x1r[:])
                for c2 in range(2):
                    tp = psum.tile([P, P], F32, tag="ps", name=f"tp{eb}_{c2}")
                    nc.tensor.transpose(tp[:], x1[:, c2 * P:(c2 + 1) * P],
                                        ident[:])
                    nc.vector.tensor_copy(
                        x1t_sb[:, c2, eb * P:(eb + 1) * P], tp[:])
            for eb in range(EBS):
                zps = psum.tile([P, CH], F32, tag="ps", name=f"ps_z{eb}")
                for c2 in range(2):
                    nc.tensor.matmul(zps[:], x1t_sb[:, c2, eb * P:(eb + 1) * P],
                                     w1bf_sb[:, c2, :], start=(c2 == 0),
                                     stop=(c2 == 1))
                zt = evac.tile([P, CH + 1], BF, tag="zt", name=f"zt{eb}")
                nc.vector.tensor_scalar_mul(zt[:, 0:CH], zps[:],
                                            ecs_sb[:, eb:eb + 1])
                nc.vector.tensor_copy(zt[:, CH:CH + 1], ecs_sb[:, eb:eb + 1])
                nc.sync.dma_start(zloc[eb * P:(eb + 1) * P, :], zt[:])

            # ---- C2: AllGather Zaug (bf16) ----
            zaug_dram = dram.tile([N_EDGES, CH + 1], BF, addr_space="Shared")
            nc.gpsimd.collective_compute(
                "AllGather", ALU.bypass, replica_groups=GROUPS,
                ins=[zloc[:].opt()], outs=[zaug_dram[:].opt()])
            nc.sync.dma_start(
                zaug_sb[:], zaug_dram.rearrange("(e p) c -> p e c", p=P))

            # ---- P3: V = B1 @ Zaug, accumulated over edge blocks ----
            with tc.tile_pool(name="btp", bufs=8) as btp:
                for nh in range(2):
                    vps = []
                    for n8 in range(8):
                        vps.append(psum.tile([P, CH + 1], F32, tag="ps",
                                             name=f"ps_v{nh}_{n8}"))
                    for ebi in range(EBF):
                        bt = btp.tile([P, 1024], BF, tag="bt",
                                      name=f"bt{nh}_{ebi}")
                        nc.sync.dma_start(
                            bt[:], b1t[ebi * P:(ebi + 1) * P,
                                       nh * 1024:(nh + 1) * 1024])
                        for n8 in range(8):
                            nc.tensor.matmul(
                                vps[n8][:], bt[:, n8 * P:(n8 + 1) * P],
                                zaug_sb[:, ebi, :],
                                start=(ebi == 0), stop=(ebi == EBF - 1))
                    for n8 in range(8):
                        n = nh * 8 + n8
                        d0i = small.tile([P, 1], F32, tag="d0i",
                                         name=f"d0i{n}")
                        nc.vector.reciprocal(d0i[:], vps[n8][:, CH:CH + 1])
                        xo = evac.tile([P, CH], F32, tag="xo", name=f"xo{n}")
                        nc.vector.tensor_scalar_mul(xo[:], vps[n8][:, 0:CH],
                                                    d0i[:])
                        nc.vector.tensor_add(xo[:], xo[:], b10_sb[:])
                        nc.vector.tensor_scalar_max(xo[:], xo[:], 0.0)
                        nc.sync.dma_start(x0o[n * P:(n + 1) * P, :], xo[:])


_NC_CACHE = None


def _get_nc():
    global _NC_CACHE
    if _NC_CACHE is None:
        _NC_CACHE = build_bass()
    return _NC_CACHE


def kernel(x_0, incidence_1, W0, W1, bias_0_to_1, bias_1_to_0):
    x_0 = np.asarray(x_0, dtype=np.float32)
    b1 = np.asarray(incidence_1, dtype=np.float32)
    W0 = np.asarray(W0, dtype=np.float32)
    W1 = np.asarray(W1, dtype=np.float32)
    b01 = np.ascontiguousarray(np.broadcast_to(
        np.asarray(bias_0_to_1, dtype=np.float32).reshape(1, CH), (P, CH)))
    b10 = np.ascontiguousarray(np.broadcast_to(
        np.asarray(bias_1_to_0, dtype=np.float32).reshape(1, CH), (P, CH)))

    b1_bf = b1.astype(BF16)
    in_maps = []
    for i in range(N_CORES):
        rows = slice(i * NS, (i + 1) * NS)
        shard = b1_bf[rows]
        in_maps.append({
            "b1r": np.ascontiguousarray(shard),
            "b1t": np.ascontiguousarray(shard.T),
            "x0t": np.ascontiguousarray(x_0[rows].T),
            "w0": W0, "w1": W1, "b01": b01, "b10": b10,
        })

    import os
    nc = _get_nc()
    trace = os.environ.get("KERNEL_TRACE", "0") != "0"
    res = run_bass_kernel_spmd(nc, in_maps, core_ids=list(range(N_CORES)),
                               trace=trace)
    if trace:
        print(f"HW exec time: {res.exec_time_ns} ns")
        print(f"trace: {res.instructions_and_trace[1] if res.instructions_and_trace else None}")
    x0_out = np.concatenate([r["x0o"] for r in res.results], axis=0)
    x1_out = np.concatenate([r["x1o"] for r in res.results], axis=0)
    return x0_out, x1_out
